# revision 6
# baseline (speedup 1.0000x reference)
"""Trainium2 Bass kernel for nn_Net_20461224198440 (topk_masking).

net: h1 = relu(x@W1+b1); h2 = relu(h1@W2+b2);
     h2 *= topk128-mask(h2); h2 *= top8-stripe-mask(stripe sums);
     h3 = relu(h2@W3+b3); out = relu(h3@W4+b4)

Design (8-way batch data parallel, 2048 rows/core):
 - L1/L2 matmuls in fp32 (top-k selection is extremely sensitive: bf16/f32r
   noise flips boundary memberships -> >1e-2 rel err). L3/L4 in float32r
   (full PE speed, ~1e-4 precision; selection already done).
 - Exact per-row top-128 of 2048 on DVE:
     prune: per-32-chunk top-8 (64x max8) -> C[128,512]  (superset of top-128
            w.p. ~1-1e-3 per row; misses recovered by the count fix-up below)
     16x (max8 + match_replace) on C -> t_C = 128th largest of C <= v128
     fix-up: count m = #{h >= t_C} - 128 extras; bottom-8 of the kept set via
            max8 on -h (exact negation); v128 = (m+1)-th smallest kept;
            final mask h >= v128 keeps exactly the top-128.
 - Stripe stage: segmented sum [128,64,32]->[128,64], max8 -> 8th sum is the
   stripe threshold, mask stripes by >=.
 - Layouts: activations for L1/L3 produced transposed (h1_T, h3_T) so they
   feed the next matmul as lhsT without extra transposes; only the masked h2
   needs an on-chip PE transpose (32 128x128 blocks per 256-row supertile).
 - W2/W3 streamed from HBM per supertile (SBUF cannot hold all fp32 weights).
"""
import numpy as np

import concourse.bass as bass
import concourse.mybir as mybir
import concourse.tile as tile
from concourse import bacc, bass_utils
from concourse.masks import make_identity

F32 = mybir.dt.float32
F32R = mybir.dt.float32r
U8 = mybir.dt.uint8
RELU = mybir.ActivationFunctionType.Relu
COPY = mybir.ActivationFunctionType.Copy
AX = mybir.AxisListType.X
OP = mybir.AluOpType

B = 16384
NCORES = 8
R = B // NCORES          # rows per core
ST = 256                 # supertile rows (f32r needs moving dim >= 256)
D_IN = 784
KP1 = 896                # 784 padded to 7*128
D_H1 = 1024
D_H2 = 2048
D_OUT = 784
NEG_BIG = -1.0e30


def _build(rows, has_b2, has_b4):
    nst = rows // ST
    nc = bacc.Bacc("TRN2", target_bir_lowering=False, debug=False)

    xt_d = nc.dram_tensor("xt", [128, 7, rows], F32, kind="ExternalInput")
    w1_d = nc.dram_tensor("w1", [128, 7, D_H1], F32, kind="ExternalInput")
    w2_d = nc.dram_tensor("w2", [128, 8, D_H2], F32, kind="ExternalInput")
    w3_d = nc.dram_tensor("w3", [128, 16, 8, 128], F32R, kind="ExternalInput")
    w4_d = nc.dram_tensor("w4", [128, 8, D_OUT], F32R, kind="ExternalInput")
    b1_d = nc.dram_tensor("b1", [128, 8], F32, kind="ExternalInput")
    b3_d = nc.dram_tensor("b3", [128, 8], F32, kind="ExternalInput")
    b2_d = nc.dram_tensor("b2", [1, D_H2], F32R, kind="ExternalInput")
    b4_d = nc.dram_tensor("b4", [1, D_OUT], F32R, kind="ExternalInput")
    out_d = nc.dram_tensor("out", [rows, D_OUT], F32, kind="ExternalOutput")

    with tile.TileContext(nc) as tc:
        with (
            tc.tile_pool(name="const", bufs=1) as constp,
            tc.tile_pool(name="w2s", bufs=4) as w2p,
            tc.tile_pool(name="w3s", bufs=2) as w3p,
            tc.tile_pool(name="xts", bufs=2) as xtp,
            tc.tile_pool(name="h1s", bufs=2) as h1p,
            tc.tile_pool(name="h2s", bufs=4) as h2p,
            tc.tile_pool(name="cands", bufs=2) as cp,
            tc.tile_pool(name="masks", bufs=2) as mkp,
            tc.tile_pool(name="scr", bufs=2) as sp,
            tc.tile_pool(name="h2ts", bufs=1) as h2tp,
            tc.tile_pool(name="h3s", bufs=1) as h3p,
            tc.tile_pool(name="outs", bufs=2) as outp,
            tc.tile_pool(name="ps13", bufs=3, space="PSUM") as p13,
            tc.tile_pool(name="ps2", bufs=2, space="PSUM") as p2,
            tc.tile_pool(name="pst", bufs=1, space="PSUM") as ptp,
            tc.tile_pool(name="ps4", bufs=1, space="PSUM") as p4p,
        ):
            # ---- constants ----
            w1_t = constp.tile([128, 7, D_H1], F32)
            nc.sync.dma_start(w1_t[:], w1_d.ap())
            w4_t = constp.tile([128, 8, D_OUT], F32R)
            nc.sync.dma_start(w4_t[:], w4_d.ap())
            b1_t = constp.tile([128, 8], F32)
            nc.sync.dma_start(b1_t[:], b1_d.ap())
            b3_t = constp.tile([128, 8], F32)
            nc.sync.dma_start(b3_t[:], b3_d.ap())
            ident = constp.tile([128, 128], F32)
            make_identity(nc, ident[:])
            misc = constp.tile([128, 16], F32)
            for c in range(8):
                nc.gpsimd.memset(misc[:, c:c + 1], float(c))
            nc.gpsimd.memset(misc[:, 8:9], NEG_BIG)
            nc.gpsimd.memset(misc[:, 9:10], 0.0)
            negbig = misc[:, 8:9]
            zero1 = misc[:, 9:10]
            iota8 = misc[:, 0:8]
            if has_b2 or has_b4:
                ones_t = constp.tile([1, 128], F32R)
                nc.gpsimd.memset(ones_t[:], 1.0)
            if has_b2:
                b2_t = constp.tile([1, D_H2], F32R)
                nc.sync.dma_start(b2_t[:], b2_d.ap())
            if has_b4:
                b4_t = constp.tile([1, D_OUT], F32R)
                nc.sync.dma_start(b4_t[:], b4_d.ap())

            for st in range(nst):
                r0 = st * ST
                # ---- L1: h1_T[m-block, rows] = relu(W1.T-block @ x_T) ----
                xt_t = xtp.tile([128, 7, ST], F32)
                nc.sync.dma_start(xt_t[:], xt_d.ap()[:, :, r0:r0 + ST])
                h1_t = h1p.tile([128, 8, ST], F32)
                for m in range(8):
                    ps = p13.tile([128, ST], F32)
                    for kc in range(7):
                        nc.tensor.matmul(
                            ps[:], w1_t[:, kc, m * 128:(m + 1) * 128],
                            xt_t[:, kc, :], start=(kc == 0), stop=(kc == 6))
                    nc.scalar.activation(h1_t[:, m, :], ps[:], RELU,
                                         bias=b1_t[:, m:m + 1])
                # ---- L2: h2[half][128, 2048] = relu(h1 @ W2) ----
                h2_halves = [h2p.tile([128, D_H2], F32, name="h2h")
                             for h in range(2)]
                for n in range(4):
                    nsl = slice(n * 512, (n + 1) * 512)
                    pss = [p2.tile([128, 512], F32, name="psL2")
                           for h in range(2)]
                    for kc in range(8):
                        w2t = w2p.tile([128, 512], F32)
                        nc.sync.dma_start(w2t[:], w2_d.ap()[:, kc, nsl])
                        for h in range(2):
                            nc.tensor.matmul(
                                pss[h][:], h1_t[:, kc, h * 128:(h + 1) * 128],
                                w2t[:], start=(kc == 0),
                                stop=(kc == 7 and not has_b2))
                    for h in range(2):
                        if has_b2:
                            nc.tensor.matmul(pss[h][:], ones_t[:],
                                             b2_t[:, nsl], start=False,
                                             stop=True)
                        nc.scalar.activation(h2_halves[h][:, nsl], pss[h][:],
                                             RELU, bias=0.0)

                # ---- top-128 masking + stripe masking, per half ----
                for h in range(2):
                    h2t = h2_halves[h]
                    C = cp.tile([128, 512], F32)
                    s = sp.tile([128, 160], F32)
                    mk = mkp.tile([128, D_H2], U8)
                    ssum = s[:, 0:64]
                    wtop = s[:, 64:72]
                    maxs = s[:, 72:80]
                    sel = s[:, 80:88]
                    cnt = s[:, 88:89]
                    m_ = s[:, 89:90]
                    v128n = s[:, 90:91]
                    # prune: per-32-chunk top-8
                    for c in range(64):
                        nc.vector.max(C[:, c * 8:(c + 1) * 8],
                                      h2t[:, c * 32:(c + 1) * 32])
                    # 16 rounds -> t_C = 128th largest of C
                    for r in range(16):
                        nc.vector.max(maxs, C[:])
                        nc.vector.match_replace(C[:], in_to_replace=maxs,
                                                in_values=C[:],
                                                imm_value=-1.0)
                    t_c = maxs[:, 7:8]
                    # mask of excluded (h < t_C) + count
                    nc.vector.tensor_scalar(mk[:], h2t[:], t_c, None,
                                            op0=OP.is_lt, op1=OP.add,
                                            accum_out=cnt)
                    # m = (2048 - cnt_lt) - 128, clamped to [_,7]
                    nc.vector.tensor_scalar(m_, cnt, -1.0, float(D_H2 - 128),
                                            op0=OP.mult, op1=OP.add)
                    nc.vector.tensor_scalar_min(m_, m_, 7.0)
                    # negate in place; excluded -> -BIG; top8 of -h = bottom8 kept
                    nc.vector.tensor_scalar_mul(h2t[:], h2t[:], -1.0)
                    nc.vector.copy_predicated(
                        h2t[:], mk[:], negbig.to_broadcast([128, D_H2]))
                    nc.vector.max(wtop, h2t[:])
                    # restore excluded to 0 (still negated space)
                    nc.vector.copy_predicated(
                        h2t[:], mk[:], zero1.to_broadcast([128, D_H2]))
                    # v128n = -v128 = wtop[m] via one-hot select
                    nc.vector.tensor_scalar(sel, iota8, m_, None,
                                            op0=OP.is_equal)
                    nc.vector.tensor_mul(sel, sel, wtop)
                    nc.vector.reduce_sum(v128n, sel, axis=AX)
                    # zap: in negated space keep where -h <= v128n i.e. flag >
                    nc.vector.tensor_scalar(mk[:], h2t[:], v128n, None,
                                            op0=OP.is_gt)
                    nc.vector.tensor_scalar_mul(h2t[:], h2t[:], -1.0)
                    nc.vector.copy_predicated(
                        h2t[:], mk[:], zero1.to_broadcast([128, D_H2]))
                    # stripes: sums over 32, top-8 of 64, mask
                    h2s3 = h2t[:].rearrange("p (s d) -> p s d", d=32)
                    nc.vector.reduce_sum(ssum, h2s3, axis=AX)
                    nc.vector.max(wtop, ssum)
                    t_s = wtop[:, 7:8]
                    smkf = s[:, 96:160]
                    nc.vector.tensor_scalar(smkf, ssum, t_s, None,
                                            op0=OP.is_ge)
                    nc.vector.tensor_mul(h2s3, h2s3,
                                         smkf.to_broadcast([128, 64, 32]))

                # ---- transpose masked h2 -> h2m_T [128, 16kc, 256] f32r ----
                h2mt = h2tp.tile([128, 16, ST], F32R)
                for h in range(2):
                    for g in range(4):
                        pt = ptp.tile([128, 512], F32)
                        for j in range(4):
                            kc = g * 4 + j
                            nc.tensor.transpose(
                                pt[:, j * 128:(j + 1) * 128],
                                h2_halves[h][:, kc * 128:(kc + 1) * 128],
                                ident[:])
                        nc.scalar.activation(
                            h2mt[:, g * 4:(g + 1) * 4, h * 128:(h + 1) * 128],
                            pt[:].rearrange("p (a b) -> p a b", b=128),
                            COPY)
                # ---- L3: h3_T[m-block, rows] = relu(W3.T-block @ h2m_T) ----
                h3_t = h3p.tile([128, 8, ST], F32R)
                for m in range(8):
                    w3t = w3p.tile([128, 16, 128], F32R)
                    nc.sync.dma_start(w3t[:], w3_d.ap()[:, :, m, :])
                    ps = p13.tile([128, ST], F32)
                    for kc in range(16):
                        nc.tensor.matmul(ps[:], w3t[:, kc, :], h2mt[:, kc, :],
                                         start=(kc == 0), stop=(kc == 15))
                    nc.scalar.activation(h3_t[:, m, :], ps[:], RELU,
                                         bias=b3_t[:, m:m + 1])
                # ---- L4: out[128, 784] = relu(h3 @ W4) ----
                for h in range(2):
                    ps4 = p4p.tile([128, 1024], F32)
                    for lo, n in ((0, 512), (512, 272)):
                        for kc in range(8):
                            nc.tensor.matmul(
                                ps4[:, lo:lo + n],
                                h3_t[:, kc, h * 128:(h + 1) * 128],
                                w4_t[:, kc, lo:lo + n], start=(kc == 0),
                                stop=(kc == 7 and not has_b4))
                        if has_b4:
                            nc.tensor.matmul(ps4[:, lo:lo + n], ones_t[:],
                                             b4_t[:, lo:lo + n], start=False,
                                             stop=True)
                    out_t = outp.tile([128, D_OUT], F32)
                    nc.scalar.activation(out_t[:], ps4[:, 0:D_OUT], RELU,
                                         bias=0.0)
                    nc.sync.dma_start(
                        out_d.ap()[r0 + h * 128:r0 + (h + 1) * 128, :],
                        out_t[:])
    nc.compile()
    return nc


_CACHE = {}


def _get_program(rows, has_b2, has_b4):
    key = (rows, has_b2, has_b4)
    if key not in _CACHE:
        _CACHE[key] = _build(rows, has_b2, has_b4)
    return _CACHE[key]


def _pack_inputs(x, W1, b1, W2, b2, W3, b3, W4, b4, rows):
    """Host-side packing; returns per-core input maps."""
    n_cores = x.shape[0] // rows
    # x -> padded, transposed, kc-major: xt[p, kc, b] = x[b, kc*128+p]
    xpad = np.zeros((x.shape[0], KP1), np.float32)
    xpad[:, :D_IN] = x
    w1p = np.zeros((KP1, D_H1), np.float32)
    w1p[:D_IN] = W1
    common = {
        "w1": np.ascontiguousarray(
            w1p.reshape(7, 128, D_H1).transpose(1, 0, 2)),
        "w2": np.ascontiguousarray(
            W2.reshape(8, 128, D_H2).transpose(1, 0, 2)),
        "w3": np.ascontiguousarray(
            W3.reshape(16, 128, 8, 128).transpose(1, 0, 2, 3)),
        "w4": np.ascontiguousarray(
            W4.reshape(8, 128, D_OUT).transpose(1, 0, 2)),
        "b1": np.ascontiguousarray(b1.reshape(8, 128).T),
        "b3": np.ascontiguousarray(b3.reshape(8, 128).T),
        "b2": b2.reshape(1, D_H2).astype(np.float32),
        "b4": b4.reshape(1, D_OUT).astype(np.float32),
    }
    in_maps = []
    for c in range(n_cores):
        xc = xpad[c * rows:(c + 1) * rows]          # [rows, 896]
        xt = np.ascontiguousarray(
            xc.T.reshape(7, 128, rows).transpose(1, 0, 2))
        in_maps.append({"xt": xt, **common})
    return in_maps


def kernel(x, W1, b1, W2, b2, W3, b3, W4, b4):
    x = np.asarray(x, np.float32)
    W1 = np.asarray(W1, np.float32); b1 = np.asarray(b1, np.float32)
    W2 = np.asarray(W2, np.float32); b2 = np.asarray(b2, np.float32)
    W3 = np.asarray(W3, np.float32); b3 = np.asarray(b3, np.float32)
    W4 = np.asarray(W4, np.float32); b4 = np.asarray(b4, np.float32)
    has_b2 = bool(np.any(b2)); has_b4 = bool(np.any(b4))
    nc = _get_program(R, has_b2, has_b4)
    in_maps = _pack_inputs(x, W1, b1, W2, b2, W3, b3, W4, b4, R)
    res = bass_utils.run_bass_kernel_spmd(nc, in_maps,
                                          core_ids=list(range(NCORES)))
    return np.concatenate([res.results[c]["out"] for c in range(NCORES)],
                          axis=0)


# revision 7
# speedup vs baseline: 1.5323x; 1.5323x over previous
"""Trainium2 Bass kernel for nn_Net_20461224198440 (topk_masking).

net: h1 = relu(x@W1+b1); h2 = relu(h1@W2+b2);
     h2 *= topk128-mask(h2); h2 *= top8-stripe-mask(stripe sums);
     h3 = relu(h2@W3+b3); out = relu(h3@W4+b4)

Design (8-way batch data parallel, 2048 rows/core):
 - L1/L2 matmuls in fp32 (top-k selection is extremely sensitive: bf16/f32r
   noise flips boundary memberships -> >1e-2 rel err). L3/L4 in float32r
   (full PE speed, ~1e-4 precision; selection already done).
 - Exact per-row top-128 of 2048 on DVE:
     prune: per-32-chunk top-8 (64x max8) -> C[128,512]  (superset of top-128
            w.p. ~1-1e-3 per row; misses recovered by the count fix-up below)
     16x (max8 + match_replace) on C -> t_C = 128th largest of C <= v128
     fix-up: count m = #{h >= t_C} - 128 extras; bottom-8 of the kept set via
            max8 on -h (exact negation); v128 = (m+1)-th smallest kept;
            final mask h >= v128 keeps exactly the top-128.
 - Stripe stage: segmented sum [128,64,32]->[128,64], max8 -> 8th sum is the
   stripe threshold, mask stripes by >=.
 - Layouts: activations for L1/L3 produced transposed (h1_T, h3_T) so they
   feed the next matmul as lhsT without extra transposes; only the masked h2
   needs an on-chip PE transpose (32 128x128 blocks per 256-row supertile).
 - W2/W3 streamed from HBM per supertile (SBUF cannot hold all fp32 weights).
"""
import numpy as np

import concourse.bass as bass
import concourse.mybir as mybir
import concourse.tile as tile
from concourse import bacc, bass_utils
from concourse.masks import make_identity

F32 = mybir.dt.float32
F32R = mybir.dt.float32r
U8 = mybir.dt.uint8
RELU = mybir.ActivationFunctionType.Relu
COPY = mybir.ActivationFunctionType.Copy
AX = mybir.AxisListType.X
OP = mybir.AluOpType

B = 16384
NCORES = 8
R = B // NCORES          # rows per core
ST = 256                 # supertile rows (f32r needs moving dim >= 256)
D_IN = 784
KP1 = 896                # 784 padded to 7*128
D_H1 = 1024
D_H2 = 2048
D_OUT = 784
NEG_BIG = -1.0e30


def _build(rows, has_b2, has_b4):
    nst = rows // ST
    nc = bacc.Bacc("TRN2", target_bir_lowering=False, debug=False)

    xt_d = nc.dram_tensor("xt", [128, 7, rows], F32, kind="ExternalInput")
    w1_d = nc.dram_tensor("w1", [128, 7, D_H1], F32, kind="ExternalInput")
    w2_d = nc.dram_tensor("w2", [128, 8, D_H2], F32, kind="ExternalInput")
    w3_d = nc.dram_tensor("w3", [128, 16, 8, 128], F32R, kind="ExternalInput")
    w4_d = nc.dram_tensor("w4", [128, 8, D_OUT], F32R, kind="ExternalInput")
    b1_d = nc.dram_tensor("b1", [128, 8], F32, kind="ExternalInput")
    b3_d = nc.dram_tensor("b3", [128, 8], F32, kind="ExternalInput")
    b2_d = nc.dram_tensor("b2", [1, D_H2], F32R, kind="ExternalInput")
    b4_d = nc.dram_tensor("b4", [1, D_OUT], F32R, kind="ExternalInput")
    out_d = nc.dram_tensor("out", [rows, D_OUT], F32, kind="ExternalOutput")

    with tile.TileContext(nc) as tc:
        with (
            tc.tile_pool(name="const", bufs=1) as constp,
            tc.tile_pool(name="w2s", bufs=4) as w2p,
            tc.tile_pool(name="w3s", bufs=2) as w3p,
            tc.tile_pool(name="xts", bufs=2) as xtp,
            tc.tile_pool(name="h1s", bufs=2) as h1p,
            tc.tile_pool(name="h2s", bufs=4) as h2p,
            tc.tile_pool(name="cands", bufs=2) as cp,
            tc.tile_pool(name="masks", bufs=2) as mkp,
            tc.tile_pool(name="scr", bufs=2) as sp,
            tc.tile_pool(name="h2ts", bufs=1) as h2tp,
            tc.tile_pool(name="h3s", bufs=1) as h3p,
            tc.tile_pool(name="outs", bufs=2) as outp,
            tc.tile_pool(name="ps13", bufs=3, space="PSUM") as p13,
            tc.tile_pool(name="ps2", bufs=2, space="PSUM") as p2,
            tc.tile_pool(name="pst", bufs=1, space="PSUM") as ptp,
            tc.tile_pool(name="ps4", bufs=1, space="PSUM") as p4p,
        ):
            # ---- constants ----
            w1_t = constp.tile([128, 7, D_H1], F32)
            nc.sync.dma_start(w1_t[:], w1_d.ap())
            w4_t = constp.tile([128, 8, D_OUT], F32R)
            nc.sync.dma_start(w4_t[:], w4_d.ap())
            b1_t = constp.tile([128, 8], F32)
            nc.sync.dma_start(b1_t[:], b1_d.ap())
            b3_t = constp.tile([128, 8], F32)
            nc.sync.dma_start(b3_t[:], b3_d.ap())
            ident = constp.tile([128, 128], F32)
            make_identity(nc, ident[:])
            misc = constp.tile([128, 16], F32)
            for c in range(8):
                nc.gpsimd.memset(misc[:, c:c + 1], float(c))
            nc.gpsimd.memset(misc[:, 8:9], NEG_BIG)
            nc.gpsimd.memset(misc[:, 9:10], 0.0)
            negbig = misc[:, 8:9]
            zero1 = misc[:, 9:10]
            iota8 = misc[:, 0:8]
            if has_b2 or has_b4:
                ones_t = constp.tile([1, 128], F32R)
                nc.gpsimd.memset(ones_t[:], 1.0)
            if has_b2:
                b2_t = constp.tile([1, D_H2], F32R)
                nc.sync.dma_start(b2_t[:], b2_d.ap())
            if has_b4:
                b4_t = constp.tile([1, D_OUT], F32R)
                nc.sync.dma_start(b4_t[:], b4_d.ap())

            h2_st = {}

            def stage_front(st):
                r0 = st * ST
                # ---- L1: h1_T[m-block, rows] = relu(W1.T-block @ x_T) ----
                xt_t = xtp.tile([128, 7, ST], F32)
                nc.sync.dma_start(xt_t[:], xt_d.ap()[:, :, r0:r0 + ST])
                h1_t = h1p.tile([128, 8, ST], F32)
                for m in range(8):
                    ps = p13.tile([128, ST], F32)
                    for kc in range(7):
                        nc.tensor.matmul(
                            ps[:], w1_t[:, kc, m * 128:(m + 1) * 128],
                            xt_t[:, kc, :], start=(kc == 0), stop=(kc == 6))
                    nc.scalar.activation(h1_t[:, m, :], ps[:], RELU,
                                         bias=b1_t[:, m:m + 1])
                # ---- L2: h2[half][128, 2048] = relu(h1 @ W2) ----
                h2_halves = [h2p.tile([128, D_H2], F32, name="h2h")
                             for h in range(2)]
                for n in range(4):
                    nsl = slice(n * 512, (n + 1) * 512)
                    pss = [p2.tile([128, 512], F32, name="psL2")
                           for h in range(2)]
                    for kc in range(8):
                        w2t = w2p.tile([128, 512], F32)
                        nc.sync.dma_start(w2t[:], w2_d.ap()[:, kc, nsl])
                        for h in range(2):
                            nc.tensor.matmul(
                                pss[h][:], h1_t[:, kc, h * 128:(h + 1) * 128],
                                w2t[:], start=(kc == 0),
                                stop=(kc == 7 and not has_b2))
                    for h in range(2):
                        if has_b2:
                            nc.tensor.matmul(pss[h][:], ones_t[:],
                                             b2_t[:, nsl], start=False,
                                             stop=True)
                        nc.scalar.activation(h2_halves[h][:, nsl], pss[h][:],
                                             RELU, bias=0.0)

                # ---- top-128 masking + stripe masking, per half ----
                for h in range(2):
                    h2t = h2_halves[h]
                    C = cp.tile([128, 512], F32)
                    s = sp.tile([128, 160], F32)
                    mk = mkp.tile([128, D_H2], U8)
                    ssum = s[:, 0:64]
                    wtop = s[:, 64:72]
                    maxs = s[:, 72:80]
                    sel = s[:, 80:88]
                    cnt = s[:, 88:89]
                    m_ = s[:, 89:90]
                    v128n = s[:, 90:91]
                    # prune: per-32-chunk top-8
                    for c in range(64):
                        nc.vector.max(C[:, c * 8:(c + 1) * 8],
                                      h2t[:, c * 32:(c + 1) * 32])
                    # 16 rounds -> t_C = 128th largest of C
                    for r in range(16):
                        nc.vector.max(maxs, C[:])
                        nc.vector.match_replace(C[:], in_to_replace=maxs,
                                                in_values=C[:],
                                                imm_value=-1.0)
                    t_c = maxs[:, 7:8]
                    # mask of excluded (h < t_C) + count
                    nc.vector.tensor_scalar(mk[:], h2t[:], t_c, None,
                                            op0=OP.is_lt, op1=OP.add,
                                            accum_out=cnt)
                    # m = (2048 - cnt_lt) - 128, clamped to [_,7]
                    nc.vector.tensor_scalar(m_, cnt, -1.0, float(D_H2 - 128),
                                            op0=OP.mult, op1=OP.add)
                    nc.vector.tensor_scalar_min(m_, m_, 7.0)
                    # negate in place; excluded -> -BIG; top8 of -h = bottom8 kept
                    nc.vector.tensor_scalar_mul(h2t[:], h2t[:], -1.0)
                    nc.vector.copy_predicated(
                        h2t[:], mk[:], negbig.to_broadcast([128, D_H2]))
                    nc.vector.max(wtop, h2t[:])
                    # restore excluded to 0 (still negated space)
                    nc.vector.copy_predicated(
                        h2t[:], mk[:], zero1.to_broadcast([128, D_H2]))
                    # v128n = -v128 = wtop[m] via one-hot select
                    nc.vector.tensor_scalar(sel, iota8, m_, None,
                                            op0=OP.is_equal)
                    nc.vector.tensor_mul(sel, sel, wtop)
                    nc.vector.reduce_sum(v128n, sel, axis=AX)
                    # zap: in negated space keep where -h <= v128n i.e. flag >
                    nc.vector.tensor_scalar(mk[:], h2t[:], v128n, None,
                                            op0=OP.is_gt)
                    nc.vector.tensor_scalar_mul(h2t[:], h2t[:], -1.0)
                    nc.vector.copy_predicated(
                        h2t[:], mk[:], zero1.to_broadcast([128, D_H2]))
                    # stripes: sums over 32, top-8 of 64, mask
                    h2s3 = h2t[:].rearrange("p (s d) -> p s d", d=32)
                    nc.vector.reduce_sum(ssum, h2s3, axis=AX)
                    nc.vector.max(wtop, ssum)
                    t_s = wtop[:, 7:8]
                    smkf = s[:, 96:160]
                    nc.vector.tensor_scalar(smkf, ssum, t_s, None,
                                            op0=OP.is_ge)
                    nc.vector.tensor_mul(h2s3, h2s3,
                                         smkf.to_broadcast([128, 64, 32]))

                h2_st[st] = h2_halves

            def stage_back(st):
                r0 = st * ST
                h2_halves = h2_st.pop(st)
                # ---- transpose masked h2 -> h2m_T [128, 16kc, 256] f32r ----
                h2mt = h2tp.tile([128, 16, ST], F32R)
                for h in range(2):
                    for g in range(4):
                        pt = ptp.tile([128, 512], F32)
                        for j in range(4):
                            kc = g * 4 + j
                            nc.tensor.transpose(
                                pt[:, j * 128:(j + 1) * 128],
                                h2_halves[h][:, kc * 128:(kc + 1) * 128],
                                ident[:])
                        nc.scalar.activation(
                            h2mt[:, g * 4:(g + 1) * 4, h * 128:(h + 1) * 128],
                            pt[:].rearrange("p (a b) -> p a b", b=128),
                            COPY)
                # ---- L3: h3_T[m-block, rows] = relu(W3.T-block @ h2m_T) ----
                h3_t = h3p.tile([128, 8, ST], F32R)
                for m in range(8):
                    w3t = w3p.tile([128, 16, 128], F32R)
                    nc.sync.dma_start(w3t[:], w3_d.ap()[:, :, m, :])
                    ps = p13.tile([128, ST], F32)
                    for kc in range(16):
                        nc.tensor.matmul(ps[:], w3t[:, kc, :], h2mt[:, kc, :],
                                         start=(kc == 0), stop=(kc == 15))
                    nc.scalar.activation(h3_t[:, m, :], ps[:], RELU,
                                         bias=b3_t[:, m:m + 1])
                # ---- L4: out[128, 784] = relu(h3 @ W4) ----
                for h in range(2):
                    ps4 = p4p.tile([128, 1024], F32)
                    for lo, n in ((0, 512), (512, 272)):
                        for kc in range(8):
                            nc.tensor.matmul(
                                ps4[:, lo:lo + n],
                                h3_t[:, kc, h * 128:(h + 1) * 128],
                                w4_t[:, kc, lo:lo + n], start=(kc == 0),
                                stop=(kc == 7 and not has_b4))
                        if has_b4:
                            nc.tensor.matmul(ps4[:, lo:lo + n], ones_t[:],
                                             b4_t[:, lo:lo + n], start=False,
                                             stop=True)
                    out_t = outp.tile([128, D_OUT], F32)
                    nc.scalar.activation(out_t[:], ps4[:, 0:D_OUT], RELU,
                                         bias=0.0)
                    nc.sync.dma_start(
                        out_d.ap()[r0 + h * 128:r0 + (h + 1) * 128, :],
                        out_t[:])

            for st in range(nst):
                stage_front(st)
                if st >= 1:
                    stage_back(st - 1)
            stage_back(nst - 1)
    nc.compile()
    return nc


_CACHE = {}


def _get_program(rows, has_b2, has_b4):
    key = (rows, has_b2, has_b4)
    if key not in _CACHE:
        _CACHE[key] = _build(rows, has_b2, has_b4)
    return _CACHE[key]


def _pack_inputs(x, W1, b1, W2, b2, W3, b3, W4, b4, rows):
    """Host-side packing; returns per-core input maps."""
    n_cores = x.shape[0] // rows
    # x -> padded, transposed, kc-major: xt[p, kc, b] = x[b, kc*128+p]
    xpad = np.zeros((x.shape[0], KP1), np.float32)
    xpad[:, :D_IN] = x
    w1p = np.zeros((KP1, D_H1), np.float32)
    w1p[:D_IN] = W1
    common = {
        "w1": np.ascontiguousarray(
            w1p.reshape(7, 128, D_H1).transpose(1, 0, 2)),
        "w2": np.ascontiguousarray(
            W2.reshape(8, 128, D_H2).transpose(1, 0, 2)),
        "w3": np.ascontiguousarray(
            W3.reshape(16, 128, 8, 128).transpose(1, 0, 2, 3)),
        "w4": np.ascontiguousarray(
            W4.reshape(8, 128, D_OUT).transpose(1, 0, 2)),
        "b1": np.ascontiguousarray(b1.reshape(8, 128).T),
        "b3": np.ascontiguousarray(b3.reshape(8, 128).T),
        "b2": b2.reshape(1, D_H2).astype(np.float32),
        "b4": b4.reshape(1, D_OUT).astype(np.float32),
    }
    in_maps = []
    for c in range(n_cores):
        xc = xpad[c * rows:(c + 1) * rows]          # [rows, 896]
        xt = np.ascontiguousarray(
            xc.T.reshape(7, 128, rows).transpose(1, 0, 2))
        in_maps.append({"xt": xt, **common})
    return in_maps


def kernel(x, W1, b1, W2, b2, W3, b3, W4, b4):
    x = np.asarray(x, np.float32)
    W1 = np.asarray(W1, np.float32); b1 = np.asarray(b1, np.float32)
    W2 = np.asarray(W2, np.float32); b2 = np.asarray(b2, np.float32)
    W3 = np.asarray(W3, np.float32); b3 = np.asarray(b3, np.float32)
    W4 = np.asarray(W4, np.float32); b4 = np.asarray(b4, np.float32)
    has_b2 = bool(np.any(b2)); has_b4 = bool(np.any(b4))
    nc = _get_program(R, has_b2, has_b4)
    in_maps = _pack_inputs(x, W1, b1, W2, b2, W3, b3, W4, b4, R)
    res = bass_utils.run_bass_kernel_spmd(nc, in_maps,
                                          core_ids=list(range(NCORES)))
    return np.concatenate([res.results[c]["out"] for c in range(NCORES)],
                          axis=0)


# revision 8
# speedup vs baseline: 1.5548x; 1.0147x over previous
"""Trainium2 Bass kernel for nn_Net_20461224198440 (topk_masking).

net: h1 = relu(x@W1+b1); h2 = relu(h1@W2+b2);
     h2 *= topk128-mask(h2); h2 *= top8-stripe-mask(stripe sums);
     h3 = relu(h2@W3+b3); out = relu(h3@W4+b4)

Design (8-way batch data parallel, 2048 rows/core):
 - L1/L2 matmuls in fp32 (top-k selection is extremely sensitive: bf16/f32r
   noise flips boundary memberships -> >1e-2 rel err). L3/L4 in float32r
   (full PE speed, ~1e-4 precision; selection already done).
 - Exact per-row top-128 of 2048 on DVE:
     prune: per-32-chunk top-8 (64x max8) -> C[128,512]  (superset of top-128
            w.p. ~1-1e-3 per row; misses recovered by the count fix-up below)
     16x (max8 + match_replace) on C -> t_C = 128th largest of C <= v128
     fix-up: count m = #{h >= t_C} - 128 extras; bottom-8 of the kept set via
            max8 on -h (exact negation); v128 = (m+1)-th smallest kept;
            final mask h >= v128 keeps exactly the top-128.
 - Stripe stage: segmented sum [128,64,32]->[128,64], max8 -> 8th sum is the
   stripe threshold, mask stripes by >=.
 - Layouts: activations for L1/L3 produced transposed (h1_T, h3_T) so they
   feed the next matmul as lhsT without extra transposes; only the masked h2
   needs an on-chip PE transpose (32 128x128 blocks per 256-row supertile).
 - W2/W3 streamed from HBM per supertile (SBUF cannot hold all fp32 weights).
"""
import numpy as np

import concourse.bass as bass
import concourse.mybir as mybir
import concourse.tile as tile
from concourse import bacc, bass_utils
from concourse.masks import make_identity

F32 = mybir.dt.float32
F32R = mybir.dt.float32r
U8 = mybir.dt.uint8
RELU = mybir.ActivationFunctionType.Relu
COPY = mybir.ActivationFunctionType.Copy
AX = mybir.AxisListType.X
OP = mybir.AluOpType

B = 16384
NCORES = 8
R = B // NCORES          # rows per core
ST = 256                 # supertile rows (f32r needs moving dim >= 256)
D_IN = 784
KP1 = 896                # 784 padded to 7*128
D_H1 = 1024
D_H2 = 2048
D_OUT = 784
NEG_BIG = -1.0e30


def _build(rows, has_b2, has_b4):
    nst = rows // ST
    nc = bacc.Bacc("TRN2", target_bir_lowering=False, debug=False)

    xt_d = nc.dram_tensor("xt", [128, 7, rows], F32, kind="ExternalInput")
    w1_d = nc.dram_tensor("w1", [128, 7, D_H1], F32, kind="ExternalInput")
    w2_d = nc.dram_tensor("w2", [128, 8, D_H2], F32, kind="ExternalInput")
    w3_d = nc.dram_tensor("w3", [128, 16, 8, 128], F32R, kind="ExternalInput")
    w4_d = nc.dram_tensor("w4", [128, 8, D_OUT], F32R, kind="ExternalInput")
    b1_d = nc.dram_tensor("b1", [128, 8], F32, kind="ExternalInput")
    b3_d = nc.dram_tensor("b3", [128, 8], F32, kind="ExternalInput")
    b2_d = nc.dram_tensor("b2", [1, D_H2], F32R, kind="ExternalInput")
    b4_d = nc.dram_tensor("b4", [1, D_OUT], F32R, kind="ExternalInput")
    out_d = nc.dram_tensor("out", [rows, D_OUT], F32, kind="ExternalOutput")

    with tile.TileContext(nc) as tc:
        with (
            tc.tile_pool(name="const", bufs=1) as constp,
            tc.tile_pool(name="w2s", bufs=4) as w2p,
            tc.tile_pool(name="w3s", bufs=2) as w3p,
            tc.tile_pool(name="xts", bufs=2) as xtp,
            tc.tile_pool(name="h1s", bufs=2) as h1p,
            tc.tile_pool(name="h2s", bufs=6) as h2p,
            tc.tile_pool(name="cands", bufs=2) as cp,
            tc.tile_pool(name="masks", bufs=2) as mkp,
            tc.tile_pool(name="scr", bufs=2) as sp,
            tc.tile_pool(name="h2ts", bufs=1) as h2tp,
            tc.tile_pool(name="h3s", bufs=1) as h3p,
            tc.tile_pool(name="outs", bufs=2) as outp,
            tc.tile_pool(name="ps13", bufs=3, space="PSUM") as p13,
            tc.tile_pool(name="ps2", bufs=2, space="PSUM") as p2,
            tc.tile_pool(name="pst", bufs=1, space="PSUM") as ptp,
            tc.tile_pool(name="ps4", bufs=1, space="PSUM") as p4p,
        ):
            # ---- constants ----
            w1_t = constp.tile([128, 7, D_H1], F32)
            nc.sync.dma_start(w1_t[:], w1_d.ap())
            w4_t = constp.tile([128, 8, D_OUT], F32R)
            nc.sync.dma_start(w4_t[:], w4_d.ap())
            b1_t = constp.tile([128, 8], F32)
            nc.sync.dma_start(b1_t[:], b1_d.ap())
            b3_t = constp.tile([128, 8], F32)
            nc.sync.dma_start(b3_t[:], b3_d.ap())
            ident = constp.tile([128, 128], F32)
            make_identity(nc, ident[:])
            misc = constp.tile([128, 16], F32)
            for c in range(8):
                nc.gpsimd.memset(misc[:, c:c + 1], float(c))
            nc.gpsimd.memset(misc[:, 8:9], NEG_BIG)
            nc.gpsimd.memset(misc[:, 9:10], 0.0)
            negbig = misc[:, 8:9]
            zero1 = misc[:, 9:10]
            iota8 = misc[:, 0:8]
            if has_b2 or has_b4:
                ones_t = constp.tile([1, 128], F32R)
                nc.gpsimd.memset(ones_t[:], 1.0)
            if has_b2:
                b2_t = constp.tile([1, D_H2], F32R)
                nc.sync.dma_start(b2_t[:], b2_d.ap())
            if has_b4:
                b4_t = constp.tile([1, D_OUT], F32R)
                nc.sync.dma_start(b4_t[:], b4_d.ap())

            h2_st = {}

            def stage_front(st):
                r0 = st * ST
                # ---- L1: h1_T[m-block, rows] = relu(W1.T-block @ x_T) ----
                xt_t = xtp.tile([128, 7, ST], F32)
                nc.sync.dma_start(xt_t[:], xt_d.ap()[:, :, r0:r0 + ST])
                h1_t = h1p.tile([128, 8, ST], F32)
                for m in range(8):
                    ps = p13.tile([128, ST], F32)
                    for kc in range(7):
                        nc.tensor.matmul(
                            ps[:], w1_t[:, kc, m * 128:(m + 1) * 128],
                            xt_t[:, kc, :], start=(kc == 0), stop=(kc == 6))
                    nc.scalar.activation(h1_t[:, m, :], ps[:], RELU,
                                         bias=b1_t[:, m:m + 1])
                # ---- L2: h2[half][128, 2048] = relu(h1 @ W2) ----
                h2_halves = [h2p.tile([128, D_H2], F32, name="h2h")
                             for h in range(2)]
                for n in range(4):
                    nsl = slice(n * 512, (n + 1) * 512)
                    pss = [p2.tile([128, 512], F32, name="psL2")
                           for h in range(2)]
                    for kc in range(8):
                        w2t = w2p.tile([128, 512], F32)
                        nc.sync.dma_start(w2t[:], w2_d.ap()[:, kc, nsl])
                        for h in range(2):
                            nc.tensor.matmul(
                                pss[h][:], h1_t[:, kc, h * 128:(h + 1) * 128],
                                w2t[:], start=(kc == 0),
                                stop=(kc == 7 and not has_b2))
                    for h in range(2):
                        if has_b2:
                            nc.tensor.matmul(pss[h][:], ones_t[:],
                                             b2_t[:, nsl], start=False,
                                             stop=True)
                        nc.scalar.activation(h2_halves[h][:, nsl], pss[h][:],
                                             RELU, bias=0.0)

                # ---- top-128 masking + stripe masking, per half ----
                for h in range(2):
                    h2t = h2_halves[h]
                    C = cp.tile([128, 512], F32)
                    s = sp.tile([128, 160], F32)
                    mk = mkp.tile([128, D_H2], U8)
                    ssum = s[:, 0:64]
                    wtop = s[:, 64:72]
                    maxs = s[:, 72:80]
                    sel = s[:, 80:88]
                    cnt = s[:, 88:89]
                    m_ = s[:, 89:90]
                    v128n = s[:, 90:91]
                    # prune: per-32-chunk top-8
                    for c in range(64):
                        nc.vector.max(C[:, c * 8:(c + 1) * 8],
                                      h2t[:, c * 32:(c + 1) * 32])
                    # 16 rounds -> t_C = 128th largest of C
                    for r in range(16):
                        nc.vector.max(maxs, C[:])
                        nc.vector.match_replace(C[:], in_to_replace=maxs,
                                                in_values=C[:],
                                                imm_value=-1.0)
                    t_c = maxs[:, 7:8]
                    # mask of excluded (h < t_C) + count
                    nc.vector.tensor_scalar(mk[:], h2t[:], t_c, None,
                                            op0=OP.is_lt, op1=OP.add,
                                            accum_out=cnt)
                    # m = (2048 - cnt_lt) - 128, clamped to [_,7]
                    nc.vector.tensor_scalar(m_, cnt, -1.0, float(D_H2 - 128),
                                            op0=OP.mult, op1=OP.add)
                    nc.vector.tensor_scalar_min(m_, m_, 7.0)
                    # negate in place; excluded -> -BIG; top8 of -h = bottom8 kept
                    nc.vector.tensor_scalar_mul(h2t[:], h2t[:], -1.0)
                    nc.vector.copy_predicated(
                        h2t[:], mk[:], negbig.to_broadcast([128, D_H2]))
                    nc.vector.max(wtop, h2t[:])
                    # restore excluded to 0 (still negated space)
                    nc.vector.copy_predicated(
                        h2t[:], mk[:], zero1.to_broadcast([128, D_H2]))
                    # v128n = -v128 = wtop[m] via one-hot select
                    nc.vector.tensor_scalar(sel, iota8, m_, None,
                                            op0=OP.is_equal)
                    nc.vector.tensor_mul(sel, sel, wtop)
                    nc.vector.reduce_sum(v128n, sel, axis=AX)
                    # zap: in negated space keep where -h <= v128n i.e. flag >
                    nc.vector.tensor_scalar(mk[:], h2t[:], v128n, None,
                                            op0=OP.is_gt)
                    nc.vector.tensor_scalar_mul(h2t[:], h2t[:], -1.0)
                    nc.vector.copy_predicated(
                        h2t[:], mk[:], zero1.to_broadcast([128, D_H2]))
                    # stripes: sums over 32, top-8 of 64, mask
                    h2s3 = h2t[:].rearrange("p (s d) -> p s d", d=32)
                    nc.vector.reduce_sum(ssum, h2s3, axis=AX)
                    nc.vector.max(wtop, ssum)
                    t_s = wtop[:, 7:8]
                    smkf = s[:, 96:160]
                    nc.vector.tensor_scalar(smkf, ssum, t_s, None,
                                            op0=OP.is_ge)
                    nc.vector.tensor_mul(h2s3, h2s3,
                                         smkf.to_broadcast([128, 64, 32]))

                h2_st[st] = h2_halves

            def stage_back(st):
                r0 = st * ST
                h2_halves = h2_st.pop(st)
                # ---- transpose masked h2 -> h2m_T [128, 16kc, 256] f32r ----
                h2mt = h2tp.tile([128, 16, ST], F32R)
                for h in range(2):
                    for g in range(4):
                        pt = ptp.tile([128, 512], F32)
                        for j in range(4):
                            kc = g * 4 + j
                            nc.tensor.transpose(
                                pt[:, j * 128:(j + 1) * 128],
                                h2_halves[h][:, kc * 128:(kc + 1) * 128],
                                ident[:])
                        nc.scalar.activation(
                            h2mt[:, g * 4:(g + 1) * 4, h * 128:(h + 1) * 128],
                            pt[:].rearrange("p (a b) -> p a b", b=128),
                            COPY)
                # ---- L3: h3_T[m-block, rows] = relu(W3.T-block @ h2m_T) ----
                h3_t = h3p.tile([128, 8, ST], F32R)
                for m in range(8):
                    w3t = w3p.tile([128, 16, 128], F32R)
                    nc.sync.dma_start(w3t[:], w3_d.ap()[:, :, m, :])
                    ps = p13.tile([128, ST], F32)
                    for kc in range(16):
                        nc.tensor.matmul(ps[:], w3t[:, kc, :], h2mt[:, kc, :],
                                         start=(kc == 0), stop=(kc == 15))
                    nc.scalar.activation(h3_t[:, m, :], ps[:], RELU,
                                         bias=b3_t[:, m:m + 1])
                # ---- L4: out[128, 784] = relu(h3 @ W4) ----
                for h in range(2):
                    ps4 = p4p.tile([128, 1024], F32)
                    for lo, n in ((0, 512), (512, 272)):
                        for kc in range(8):
                            nc.tensor.matmul(
                                ps4[:, lo:lo + n],
                                h3_t[:, kc, h * 128:(h + 1) * 128],
                                w4_t[:, kc, lo:lo + n], start=(kc == 0),
                                stop=(kc == 7 and not has_b4))
                        if has_b4:
                            nc.tensor.matmul(ps4[:, lo:lo + n], ones_t[:],
                                             b4_t[:, lo:lo + n], start=False,
                                             stop=True)
                    out_t = outp.tile([128, D_OUT], F32)
                    nc.scalar.activation(out_t[:], ps4[:, 0:D_OUT], RELU,
                                         bias=0.0)
                    nc.sync.dma_start(
                        out_d.ap()[r0 + h * 128:r0 + (h + 1) * 128, :],
                        out_t[:])

            for st in range(nst):
                stage_front(st)
                if st >= 1:
                    stage_back(st - 1)
            stage_back(nst - 1)
    nc.compile()
    return nc


_CACHE = {}


def _get_program(rows, has_b2, has_b4):
    key = (rows, has_b2, has_b4)
    if key not in _CACHE:
        _CACHE[key] = _build(rows, has_b2, has_b4)
    return _CACHE[key]


def _pack_inputs(x, W1, b1, W2, b2, W3, b3, W4, b4, rows):
    """Host-side packing; returns per-core input maps."""
    n_cores = x.shape[0] // rows
    # x -> padded, transposed, kc-major: xt[p, kc, b] = x[b, kc*128+p]
    xpad = np.zeros((x.shape[0], KP1), np.float32)
    xpad[:, :D_IN] = x
    w1p = np.zeros((KP1, D_H1), np.float32)
    w1p[:D_IN] = W1
    common = {
        "w1": np.ascontiguousarray(
            w1p.reshape(7, 128, D_H1).transpose(1, 0, 2)),
        "w2": np.ascontiguousarray(
            W2.reshape(8, 128, D_H2).transpose(1, 0, 2)),
        "w3": np.ascontiguousarray(
            W3.reshape(16, 128, 8, 128).transpose(1, 0, 2, 3)),
        "w4": np.ascontiguousarray(
            W4.reshape(8, 128, D_OUT).transpose(1, 0, 2)),
        "b1": np.ascontiguousarray(b1.reshape(8, 128).T),
        "b3": np.ascontiguousarray(b3.reshape(8, 128).T),
        "b2": b2.reshape(1, D_H2).astype(np.float32),
        "b4": b4.reshape(1, D_OUT).astype(np.float32),
    }
    in_maps = []
    for c in range(n_cores):
        xc = xpad[c * rows:(c + 1) * rows]          # [rows, 896]
        xt = np.ascontiguousarray(
            xc.T.reshape(7, 128, rows).transpose(1, 0, 2))
        in_maps.append({"xt": xt, **common})
    return in_maps


def kernel(x, W1, b1, W2, b2, W3, b3, W4, b4):
    x = np.asarray(x, np.float32)
    W1 = np.asarray(W1, np.float32); b1 = np.asarray(b1, np.float32)
    W2 = np.asarray(W2, np.float32); b2 = np.asarray(b2, np.float32)
    W3 = np.asarray(W3, np.float32); b3 = np.asarray(b3, np.float32)
    W4 = np.asarray(W4, np.float32); b4 = np.asarray(b4, np.float32)
    has_b2 = bool(np.any(b2)); has_b4 = bool(np.any(b4))
    nc = _get_program(R, has_b2, has_b4)
    in_maps = _pack_inputs(x, W1, b1, W2, b2, W3, b3, W4, b4, R)
    res = bass_utils.run_bass_kernel_spmd(nc, in_maps,
                                          core_ids=list(range(NCORES)))
    return np.concatenate([res.results[c]["out"] for c in range(NCORES)],
                          axis=0)
